# revision 27
# baseline (speedup 1.0000x reference)
"""Trainium2 Bass kernel for ConcatHandshaking.

out[b, p, :] = tanh(hidden[b, i_p] @ W1.T + hidden[b, j_p] @ W2.T + fc_b)
for the S*(S+1)/2 upper-triangular pairs (i_p, j_p), i-major order.

Device layout: output features (H=768) on SBUF partitions, pair index on the
free dim.  The pair dimension is emitted DIAGONAL-major: for diagonal
d = j - i, out(:, i, i+d) = p1T[:, i] + q2T[:, i+d] is an elementwise add of
two contiguous windows -- no broadcast operand.  G consecutive diagonals are
blocked into ONE DVE tensor_tensor via a 3D access pattern (row g reads
p1[0:L] and q2[d0+g : d0+g+L]); rows keep the max length L = 256-d0, so row
g writes g pad columns at its tail.  Everything (p1, q2, adds, tanh, output
DMA) runs in bf16: the adds qualify for the DVE 2x packed mode and the
output DMA halves vs f32.  The host drops pad columns and restores triu
order with one precomputed gather, then converts to f32.

The wall is the Scalar (ACT) engine: every output element needs one tanh
and ACT is the only engine with activations, so exec ~= lead-in + 91us of
gap-free ACT + DMA drain.  Lead-in optimizations: the input tensor is
column-grouped [ht | w_c0 | w_c1 | w_c2] so three small DMAs deliver
exactly what stripe 0's matmuls need first; stripe c+1's matmuls carry a
nosync dep on stripe c's PSUM stops so the scheduler cannot interleave all
stripes k-tile-major (which would delay stripe 0's PSUM -> first tanh by
~5us); the first four blocks are chunked alone so the first tanh + output
DMA launch early and ACT absorbs DVE's catch-up without a bubble.  The
trailing chunks are split small so the final output DMA drains fast, and
the very last one issues from the ACT sequencer.

Sharding (8 cores): core k handles batch b = k//2 and output-feature rows
[384*(k%2), 384*(k%2)+384) -> 3 stripes of [128 features, PPAD cols] each.
"""

import sys

import numpy as np

for _p in ("/opt/trn_rl_repo",):
    if _p not in sys.path:
        sys.path.insert(0, _p)

B, S, H = 4, 256, 768
P = S * (S + 1) // 2  # 32896
KT = H // 128  # 6 k-tiles
OC = 3  # o-chunks (of 128) per core
# bf16 packed matmul input columns: [ ht (S) | w1_c0 w2_c0 | w1_c1 w2_c1 | ... ]
IC16 = S + 2 * 128 * OC  # 1024

GPAD = 8  # q2 pad columns (max G)
# diagonal blocks (d0, G, L).  Leaders are tiny so the first tanh fires
# early; the long diagonals use G=8 (few DVE ops); the short tail uses G=4
# (pad G(G-1)/2 per block would be up to 44% of a small block at G=8 --
# every pad column costs tanh time on the ACT bottleneck, while the extra
# DVE instructions fit in DVE's ~25us of slack).
BLOCKS = (
    [(0, 1, 256), (1, 1, 255), (2, 2, 254), (4, 4, 252)]
    + [(8 + 8 * t, 8, 248 - 8 * t) for t in range(15)]
    + [(128 + 4 * u, 4, 128 - 4 * u) for u in range(32)]
)
_bases = np.concatenate([[0], np.cumsum([g * l for (_, g, l) in BLOCKS])])
BLK_BASE = _bases.astype(np.int64)
PPAD = int(BLK_BASE[-1])  # 33776

TARGET = 3500  # chunk col target for ACT + output DMA granularity


def _chunks():
    """(block_lo, block_hi, col_off, n_cols) groups; first two chunks are the
    single G=4 blocks (early first DMA), last chunk split small (short drain)."""
    chunks = [
        (b, b + 1, int(BLK_BASE[b]), int(BLK_BASE[b + 1] - BLK_BASE[b]))
        for b in range(4)
    ]
    b = 4
    while b < len(BLOCKS):
        e = b + 1
        while e < len(BLOCKS) and BLK_BASE[e] - BLK_BASE[b] < TARGET:
            e += 1
        chunks.append((b, e, int(BLK_BASE[b]), int(BLK_BASE[e] - BLK_BASE[b])))
        b = e
    # split the trailing chunk while it is large, so the final output DMA
    # (the pipeline drain) moves little data
    while chunks[-1][3] > 1000:
        blo, bhi, coff, csz = chunks.pop()
        mid = blo + 1
        while BLK_BASE[mid] - BLK_BASE[blo] < csz // 2:
            mid += 1
        chunks.append((blo, mid, coff, int(BLK_BASE[mid] - BLK_BASE[blo])))
        chunks.append((mid, bhi, int(BLK_BASE[mid]), int(BLK_BASE[bhi] - BLK_BASE[mid])))
    return chunks


CHUNKS = _chunks()
CMAX = max(c[3] for c in CHUNKS)

_NC_CACHE = {}
LAST = {}


def _build_nc():
    import bass_rust
    import concourse.bacc as bacc
    import concourse.bass as bass
    import concourse.mybir as mybir
    import concourse.tile as tile

    def _sub_ap(t, off, dims):
        return bass.AP(tensor=t.tensor, offset=t.offset + off, ap=[t.ap[0]] + dims)

    f32 = mybir.dt.float32
    bf16 = mybir.dt.bfloat16
    nc = bacc.Bacc()

    inp16_d = nc.declare_dram_parameter("inp16", [H, IC16], bf16, isOutput=False)
    # f32 side data: col 0 = fcb (rows 0:384), col 1 = zeros
    aux_d = nc.declare_dram_parameter("aux", [H, 2], f32, isOutput=False)
    out_d = nc.declare_dram_parameter("out", [OC, 128, PPAD], bf16, isOutput=True)

    Tanh = mybir.ActivationFunctionType.Tanh

    with tile.TileContext(nc) as tc:
        with (
            tc.tile_pool(name="const", bufs=1) as cpool,
            tc.tile_pool(name="mm", bufs=3, space="PSUM") as mpool,
            tc.tile_pool(name="outp", bufs=6) as opool,
            tc.tile_pool(name="outp2", bufs=6) as opool2,
        ):
            inp_b = cpool.tile([128, KT * IC16], bf16, name="inp_b")
            inp_r = inp_b[:].rearrange("p (t c) -> p t c", t=KT)
            src_r = inp16_d.rearrange("(t p) c -> p t c", p=128)
            # part A: ht + stripe-0 weights, one k-tile per DMA, issues
            # alternating between the SP and DVE sequencers (a single
            # sequencer serializes issues ~0.7us apart and stalls PE)
            for kk in range(KT):
                eng = nc.sync if kk % 2 == 0 else nc.scalar
                eng.dma_start(
                    inp_r[:, kk : kk + 1, 0:512], src_r[:, kk : kk + 1, 0:512]
                )
            aux_b = cpool.tile([128, KT * 2], f32, name="aux_b")
            nc.sync.dma_start(
                aux_b[:].rearrange("p (t c) -> p t c", t=KT),
                aux_d.rearrange("(t p) c -> p t c", p=128),
            )
            # part B: stripe 1-2 weights, one DMA
            nc.sync.dma_start(
                inp_r[:, :, 512:IC16], src_r[:, :, 512:IC16]
            )

            ht_t = [inp_b[:, kk * IC16 : kk * IC16 + S] for kk in range(KT)]
            fcb_t = [aux_b[:, c * 2 : c * 2 + 1] for c in range(OC)]

            prev_stops = []
            for c in range(OC):
                w1c = S + 256 * c
                w2c = S + 256 * c + 128
                pm1 = mpool.tile([128, S], f32, name="pm1")
                pm2 = mpool.tile([128, S], f32, name="pm2")
                stops = []
                for pm, wc in ((pm1, w1c), (pm2, w2c)):
                    for kk in range(KT):
                        mm = nc.tensor.matmul(
                            pm[:, :S],
                            inp_b[:, kk * IC16 + wc : kk * IC16 + wc + 128],
                            ht_t[kk],
                            start=(kk == 0),
                            stop=(kk == KT - 1),
                        )
                        if kk == 0 and prev_stops:
                            # keep PE stripe-major: without this the scheduler
                            # interleaves all stripes k-tile-major and stripe
                            # 0's PSUM stop retires ~5us late
                            deps = bass_rust.InstructionNameOrderedSet()
                            for nm in prev_stops:
                                deps.add(nm)
                            mm.ins.add_nosync_dependencies_from(deps)
                        if kk == KT - 1:
                            stops.append(mm.ins.name)
                prev_stops = stops

                p1 = cpool.tile([128, S], bf16, name=f"p1_{c}")
                q2 = cpool.tile([128, S + GPAD], bf16, name=f"q2_{c}")
                nc.vector.memset(q2[:, S : S + GPAD], 0.0)
                nc.vector.tensor_copy(p1[:], pm1[:])
                nc.vector.tensor_scalar_add(q2[:, :S], pm2[:], fcb_t[c])
                for ci, (blo, bhi, coff, csz) in enumerate(CHUNKS):
                    ot = opool.tile([128, CMAX], bf16, name="ot")
                    for bb in range(blo, bhi):
                        d0, G, L = BLOCKS[bb]
                        off = int(BLK_BASE[bb]) - coff
                        nc.vector.tensor_tensor(
                            _sub_ap(ot, off, [[L, G], [1, L]]),
                            _sub_ap(p1, 0, [[0, G], [1, L]]),
                            _sub_ap(q2, d0, [[1, G], [1, L]]),
                            op=mybir.AluOpType.add,
                        )
                    ot2 = opool2.tile([128, CMAX], bf16, name="ot2")
                    nc.scalar.activation(ot2[:, :csz], ot[:, :csz], Tanh)
                    # the final three DMAs issue from three different
                    # sequencers (ACT/Pool idle by then) so their issue+DGE
                    # latencies overlap instead of stacking on SP's queue
                    dma_eng = nc.sync
                    if c == OC - 1:
                        if ci == len(CHUNKS) - 1:
                            dma_eng = nc.scalar
                    dma_eng.dma_start(out_d[c, :, coff : coff + csz], ot2[:, :csz])
    nc.compile()
    return nc


def _get_nc():
    if "nc" not in _NC_CACHE:
        _NC_CACHE["nc"] = _build_nc()
    return _NC_CACHE["nc"]


def _make_in_maps(hidden_state, fc_w, fc_b):
    import ml_dtypes

    in_maps = []
    for k in range(8):
        b, h0 = k // 2, 384 * (k % 2)
        inp16 = np.empty((H, IC16), dtype=ml_dtypes.bfloat16)
        inp16[:, :S] = hidden_state[b].T.astype(ml_dtypes.bfloat16)
        for c in range(OC):
            r0 = h0 + 128 * c
            inp16[:, S + 256 * c : S + 256 * c + 128] = fc_w[
                r0 : r0 + 128, :H
            ].T.astype(ml_dtypes.bfloat16)
            inp16[:, S + 256 * c + 128 : S + 256 * c + 256] = fc_w[
                r0 : r0 + 128, H:
            ].T.astype(ml_dtypes.bfloat16)
        aux = np.zeros((H, 2), dtype=np.float32)
        aux[: 128 * OC, 0] = fc_b[h0 : h0 + 384]
        in_maps.append(dict(inp16=inp16, aux=aux))
    return in_maps


def _devcol():
    """Map triu pair index p -> device (diagonal-major padded) column."""
    colstart = np.empty(S, dtype=np.int64)
    for bi, (d0, G, L) in enumerate(BLOCKS):
        for g in range(G):
            colstart[d0 + g] = BLK_BASE[bi] + g * L
    ii, jj = np.triu_indices(S)
    return colstart[jj - ii] + ii


_DEVCOL = _devcol()


def kernel(hidden_state, fc_w, fc_b, _trace=False, **_trace_kwargs):
    from concourse.bass_utils import run_bass_kernel_spmd

    hidden_state = np.asarray(hidden_state, dtype=np.float32)
    fc_w = np.asarray(fc_w, dtype=np.float32)
    fc_b = np.asarray(fc_b, dtype=np.float32)

    in_maps = _make_in_maps(hidden_state, fc_w, fc_b)
    nc = _get_nc()
    res = run_bass_kernel_spmd(
        nc, in_maps, core_ids=list(range(8)), trace=_trace, **_trace_kwargs
    )
    LAST["res"] = res

    full = np.empty((B, H, P), dtype=np.float32)
    for k in range(8):
        b, h0 = k // 2, 384 * (k % 2)
        dev = res.results[k]["out"].reshape(384, PPAD)
        full[b, h0 : h0 + 384] = dev[:, _DEVCOL].astype(np.float32)
    return np.ascontiguousarray(full.transpose(0, 2, 1))


# revision 28
# speedup vs baseline: 1.0178x; 1.0178x over previous
"""Trainium2 Bass kernel for ConcatHandshaking.

out[b, p, :] = tanh(hidden[b, i_p] @ W1.T + hidden[b, j_p] @ W2.T + fc_b)
for the S*(S+1)/2 upper-triangular pairs (i_p, j_p), i-major order.

Device layout: output features (H=768) on SBUF partitions, pair index on the
free dim.  The pair dimension is emitted DIAGONAL-major: for diagonal
d = j - i, out(:, i, i+d) = p1T[:, i] + q2T[:, i+d] is an elementwise add of
two contiguous windows -- no broadcast operand.  G consecutive diagonals are
blocked into ONE DVE tensor_tensor via a 3D access pattern (row g reads
p1[0:L] and q2[d0+g : d0+g+L]); rows keep the max length L = 256-d0, so row
g writes g pad columns at its tail.  Everything (p1, q2, adds, tanh, output
DMA) runs in bf16: the adds qualify for the DVE 2x packed mode and the
output DMA halves vs f32.  The host drops pad columns and restores triu
order with one precomputed gather, then converts to f32.

The wall is the Scalar (ACT) engine: every output element needs one tanh
and ACT is the only engine with activations, so exec ~= lead-in + 91us of
gap-free ACT + DMA drain.  Lead-in optimizations: the input tensor is
column-grouped [ht | w_c0 | w_c1 | w_c2] so three small DMAs deliver
exactly what stripe 0's matmuls need first; stripe c+1's matmuls carry a
nosync dep on stripe c's PSUM stops so the scheduler cannot interleave all
stripes k-tile-major (which would delay stripe 0's PSUM -> first tanh by
~5us); the first four blocks are chunked alone so the first tanh + output
DMA launch early and ACT absorbs DVE's catch-up without a bubble.  The
trailing chunks are split small so the final output DMA drains fast, and
the very last one issues from the ACT sequencer.

Sharding (8 cores): core k handles batch b = k//2 and output-feature rows
[384*(k%2), 384*(k%2)+384) -> 3 stripes of [128 features, PPAD cols] each.
"""

import sys

import numpy as np

for _p in ("/opt/trn_rl_repo",):
    if _p not in sys.path:
        sys.path.insert(0, _p)

B, S, H = 4, 256, 768
P = S * (S + 1) // 2  # 32896
KT = H // 128  # 6 k-tiles
OC = 3  # o-chunks (of 128) per core
# bf16 packed matmul input columns: [ ht (S) | w1_c0 w2_c0 | w1_c1 w2_c1 | ... ]
IC16 = S + 2 * 128 * OC  # 1024

GPAD = 8  # q2 pad columns (max G)
# diagonal blocks (d0, G, L).  Leaders are tiny so the first tanh fires
# early; everything else uses G=8.  (G=4 on the short tail was tried to
# halve pad columns: the extra ~0.3us/instruction DVE time makes tail-chunk
# production slower than ACT consumption and ACT stalls ~2.4us per stripe
# tail -- worse than the 0.7us of pad it saves.)
BLOCKS = (
    [(0, 1, 256), (1, 1, 255), (2, 2, 254), (4, 4, 252)]
    + [(8 + 8 * t, 8, 248 - 8 * t) for t in range(31)]
)
_bases = np.concatenate([[0], np.cumsum([g * l for (_, g, l) in BLOCKS])])
BLK_BASE = _bases.astype(np.int64)
PPAD = int(BLK_BASE[-1])  # 33776

TARGET = 3500  # chunk col target for ACT + output DMA granularity


def _chunks():
    """(block_lo, block_hi, col_off, n_cols) groups; first two chunks are the
    single G=4 blocks (early first DMA), last chunk split small (short drain)."""
    chunks = [
        (b, b + 1, int(BLK_BASE[b]), int(BLK_BASE[b + 1] - BLK_BASE[b]))
        for b in range(4)
    ]
    b = 4
    while b < len(BLOCKS):
        e = b + 1
        while e < len(BLOCKS) and BLK_BASE[e] - BLK_BASE[b] < TARGET:
            e += 1
        chunks.append((b, e, int(BLK_BASE[b]), int(BLK_BASE[e] - BLK_BASE[b])))
        b = e
    # split the trailing chunk while it is large, so the final output DMA
    # (the pipeline drain) moves little data
    while chunks[-1][3] > 1000:
        blo, bhi, coff, csz = chunks.pop()
        mid = blo + 1
        while BLK_BASE[mid] - BLK_BASE[blo] < csz // 2:
            mid += 1
        chunks.append((blo, mid, coff, int(BLK_BASE[mid] - BLK_BASE[blo])))
        chunks.append((mid, bhi, int(BLK_BASE[mid]), int(BLK_BASE[bhi] - BLK_BASE[mid])))
    return chunks


CHUNKS = _chunks()
CMAX = max(c[3] for c in CHUNKS)

_NC_CACHE = {}
LAST = {}


def _build_nc():
    import bass_rust
    import concourse.bacc as bacc
    import concourse.bass as bass
    import concourse.mybir as mybir
    import concourse.tile as tile

    def _sub_ap(t, off, dims):
        return bass.AP(tensor=t.tensor, offset=t.offset + off, ap=[t.ap[0]] + dims)

    f32 = mybir.dt.float32
    bf16 = mybir.dt.bfloat16
    nc = bacc.Bacc()

    inp16_d = nc.declare_dram_parameter("inp16", [H, IC16], bf16, isOutput=False)
    # f32 side data: col 0 = fcb (rows 0:384), col 1 = zeros
    aux_d = nc.declare_dram_parameter("aux", [H, 2], f32, isOutput=False)
    out_d = nc.declare_dram_parameter("out", [OC, 128, PPAD], bf16, isOutput=True)

    Tanh = mybir.ActivationFunctionType.Tanh

    with tile.TileContext(nc) as tc:
        with (
            tc.tile_pool(name="const", bufs=1) as cpool,
            tc.tile_pool(name="mm", bufs=3, space="PSUM") as mpool,
            tc.tile_pool(name="outp", bufs=6) as opool,
            tc.tile_pool(name="outp2", bufs=6) as opool2,
        ):
            inp_b = cpool.tile([128, KT * IC16], bf16, name="inp_b")
            inp_r = inp_b[:].rearrange("p (t c) -> p t c", t=KT)
            src_r = inp16_d.rearrange("(t p) c -> p t c", p=128)
            # part A: ht + stripe-0 weights, one k-tile per DMA, issues
            # alternating between the SP and DVE sequencers (a single
            # sequencer serializes issues ~0.7us apart and stalls PE)
            for kk in range(KT):
                eng = nc.sync if kk % 2 == 0 else nc.scalar
                eng.dma_start(
                    inp_r[:, kk : kk + 1, 0:512], src_r[:, kk : kk + 1, 0:512]
                )
            aux_b = cpool.tile([128, KT * 2], f32, name="aux_b")
            nc.sync.dma_start(
                aux_b[:].rearrange("p (t c) -> p t c", t=KT),
                aux_d.rearrange("(t p) c -> p t c", p=128),
            )
            # part B: stripe 1-2 weights, one DMA
            nc.sync.dma_start(
                inp_r[:, :, 512:IC16], src_r[:, :, 512:IC16]
            )

            ht_t = [inp_b[:, kk * IC16 : kk * IC16 + S] for kk in range(KT)]
            fcb_t = [aux_b[:, c * 2 : c * 2 + 1] for c in range(OC)]

            prev_stops = []
            for c in range(OC):
                w1c = S + 256 * c
                w2c = S + 256 * c + 128
                pm1 = mpool.tile([128, S], f32, name="pm1")
                pm2 = mpool.tile([128, S], f32, name="pm2")
                stops = []
                for pm, wc in ((pm1, w1c), (pm2, w2c)):
                    for kk in range(KT):
                        mm = nc.tensor.matmul(
                            pm[:, :S],
                            inp_b[:, kk * IC16 + wc : kk * IC16 + wc + 128],
                            ht_t[kk],
                            start=(kk == 0),
                            stop=(kk == KT - 1),
                        )
                        if kk == 0 and prev_stops:
                            # keep PE stripe-major: without this the scheduler
                            # interleaves all stripes k-tile-major and stripe
                            # 0's PSUM stop retires ~5us late
                            deps = bass_rust.InstructionNameOrderedSet()
                            for nm in prev_stops:
                                deps.add(nm)
                            mm.ins.add_nosync_dependencies_from(deps)
                        if kk == KT - 1:
                            stops.append(mm.ins.name)
                prev_stops = stops

                p1 = cpool.tile([128, S], bf16, name=f"p1_{c}")
                q2 = cpool.tile([128, S + GPAD], bf16, name=f"q2_{c}")
                nc.vector.memset(q2[:, S : S + GPAD], 0.0)
                nc.vector.tensor_copy(p1[:], pm1[:])
                nc.vector.tensor_scalar_add(q2[:, :S], pm2[:], fcb_t[c])
                for ci, (blo, bhi, coff, csz) in enumerate(CHUNKS):
                    ot = opool.tile([128, CMAX], bf16, name="ot")
                    for bb in range(blo, bhi):
                        d0, G, L = BLOCKS[bb]
                        off = int(BLK_BASE[bb]) - coff
                        nc.vector.tensor_tensor(
                            _sub_ap(ot, off, [[L, G], [1, L]]),
                            _sub_ap(p1, 0, [[0, G], [1, L]]),
                            _sub_ap(q2, d0, [[1, G], [1, L]]),
                            op=mybir.AluOpType.add,
                        )
                    ot2 = opool2.tile([128, CMAX], bf16, name="ot2")
                    nc.scalar.activation(ot2[:, :csz], ot[:, :csz], Tanh)
                    # the final three DMAs issue from three different
                    # sequencers (ACT/Pool idle by then) so their issue+DGE
                    # latencies overlap instead of stacking on SP's queue
                    dma_eng = nc.sync
                    if c == OC - 1:
                        if ci == len(CHUNKS) - 1:
                            dma_eng = nc.scalar
                    dma_eng.dma_start(out_d[c, :, coff : coff + csz], ot2[:, :csz])
    nc.compile()
    return nc


def _get_nc():
    if "nc" not in _NC_CACHE:
        _NC_CACHE["nc"] = _build_nc()
    return _NC_CACHE["nc"]


def _make_in_maps(hidden_state, fc_w, fc_b):
    import ml_dtypes

    in_maps = []
    for k in range(8):
        b, h0 = k // 2, 384 * (k % 2)
        inp16 = np.empty((H, IC16), dtype=ml_dtypes.bfloat16)
        inp16[:, :S] = hidden_state[b].T.astype(ml_dtypes.bfloat16)
        for c in range(OC):
            r0 = h0 + 128 * c
            inp16[:, S + 256 * c : S + 256 * c + 128] = fc_w[
                r0 : r0 + 128, :H
            ].T.astype(ml_dtypes.bfloat16)
            inp16[:, S + 256 * c + 128 : S + 256 * c + 256] = fc_w[
                r0 : r0 + 128, H:
            ].T.astype(ml_dtypes.bfloat16)
        aux = np.zeros((H, 2), dtype=np.float32)
        aux[: 128 * OC, 0] = fc_b[h0 : h0 + 384]
        in_maps.append(dict(inp16=inp16, aux=aux))
    return in_maps


def _devcol():
    """Map triu pair index p -> device (diagonal-major padded) column."""
    colstart = np.empty(S, dtype=np.int64)
    for bi, (d0, G, L) in enumerate(BLOCKS):
        for g in range(G):
            colstart[d0 + g] = BLK_BASE[bi] + g * L
    ii, jj = np.triu_indices(S)
    return colstart[jj - ii] + ii


_DEVCOL = _devcol()


def kernel(hidden_state, fc_w, fc_b, _trace=False, **_trace_kwargs):
    from concourse.bass_utils import run_bass_kernel_spmd

    hidden_state = np.asarray(hidden_state, dtype=np.float32)
    fc_w = np.asarray(fc_w, dtype=np.float32)
    fc_b = np.asarray(fc_b, dtype=np.float32)

    in_maps = _make_in_maps(hidden_state, fc_w, fc_b)
    nc = _get_nc()
    res = run_bass_kernel_spmd(
        nc, in_maps, core_ids=list(range(8)), trace=_trace, **_trace_kwargs
    )
    LAST["res"] = res

    full = np.empty((B, H, P), dtype=np.float32)
    for k in range(8):
        b, h0 = k // 2, 384 * (k % 2)
        dev = res.results[k]["out"].reshape(384, PPAD)
        full[b, h0 : h0 + 384] = dev[:, _DEVCOL].astype(np.float32)
    return np.ascontiguousarray(full.transpose(0, 2, 1))


# revision 29
# speedup vs baseline: 1.0599x; 1.0413x over previous
"""Trainium2 Bass kernel for ConcatHandshaking.

out[b, p, :] = tanh(hidden[b, i_p] @ W1.T + hidden[b, j_p] @ W2.T + fc_b)
for the S*(S+1)/2 upper-triangular pairs (i_p, j_p), i-major order.

Device layout: output features (H=768) on SBUF partitions, pair index on the
free dim.  The pair dimension is emitted DIAGONAL-major: for diagonal
d = j - i, out(:, i, i+d) = p1T[:, i] + q2T[:, i+d] is an elementwise add of
two contiguous windows -- no broadcast operand.  G consecutive diagonals are
blocked into ONE DVE tensor_tensor via a 3D access pattern (row g reads
p1[0:L] and q2[d0+g : d0+g+L]); rows keep the max length L = 256-d0, so row
g writes g pad columns at its tail.  Everything (p1, q2, adds, tanh, output
DMA) runs in bf16: the adds qualify for the DVE 2x packed mode and the
output DMA halves vs f32.  The host drops pad columns and restores triu
order with one precomputed gather, then converts to f32.

The wall is the Scalar (ACT) engine: every output element needs one tanh
and ACT is the only engine with activations, so exec ~= lead-in + 91us of
gap-free ACT + DMA drain.  Lead-in optimizations: the input tensor is
column-grouped [ht | w_c0 | w_c1 | w_c2] so three small DMAs deliver
exactly what stripe 0's matmuls need first; stripe c+1's matmuls carry a
nosync dep on stripe c's PSUM stops so the scheduler cannot interleave all
stripes k-tile-major (which would delay stripe 0's PSUM -> first tanh by
~5us); the first four blocks are chunked alone so the first tanh + output
DMA launch early and ACT absorbs DVE's catch-up without a bubble.  The
trailing chunks are split small so the final output DMA drains fast, and
the very last one issues from the ACT sequencer.

Sharding (8 cores): core k handles batch b = k//2 and output-feature rows
[384*(k%2), 384*(k%2)+384) -> 3 stripes of [128 features, PPAD cols] each.
"""

import sys

import numpy as np

for _p in ("/opt/trn_rl_repo",):
    if _p not in sys.path:
        sys.path.insert(0, _p)

B, S, H = 4, 256, 768
P = S * (S + 1) // 2  # 32896
KT = H // 128  # 6 k-tiles
OC = 3  # o-chunks (of 128) per core
# bf16 packed matmul input columns: [ ht (S) | w1_c0 w2_c0 | w1_c1 w2_c1 | ... ]
IC16 = S + 2 * 128 * OC  # 1024

GPAD = 8  # q2 pad columns (max G)
# diagonal blocks (d0, G, L): two G=4 leaders for a fast first tanh, then
# G=8.  Tried and rejected: G=4 on the short tail (halves pad cols but the
# extra ~0.3us/instruction DVE time makes tail-chunk production slower than
# ACT consumption -> ACT stalls ~2.4us per stripe tail); single-diagonal
# leader chunks (each sub-1k tanh pays ~250ns ACT init and the longer
# stripe-boundary DVE chain eats the lookahead cushion).
BLOCKS = [(0, 4, 256), (4, 4, 252)] + [
    (8 + 8 * t, 8, 248 - 8 * t) for t in range(31)
]
_bases = np.concatenate([[0], np.cumsum([g * l for (_, g, l) in BLOCKS])])
BLK_BASE = _bases.astype(np.int64)
PPAD = int(BLK_BASE[-1])  # 33776

TARGET = 3500  # chunk col target for ACT + output DMA granularity


def _chunks():
    """(block_lo, block_hi, col_off, n_cols) groups; first two chunks are the
    single G=4 blocks (early first DMA), last chunk split small (short drain)."""
    chunks = [
        (b, b + 1, int(BLK_BASE[b]), int(BLK_BASE[b + 1] - BLK_BASE[b]))
        for b in range(4)
    ]
    b = 4
    while b < len(BLOCKS):
        e = b + 1
        while e < len(BLOCKS) and BLK_BASE[e] - BLK_BASE[b] < TARGET:
            e += 1
        chunks.append((b, e, int(BLK_BASE[b]), int(BLK_BASE[e] - BLK_BASE[b])))
        b = e
    # split the trailing chunk while it is large, so the final output DMA
    # (the pipeline drain) moves little data
    while chunks[-1][3] > 1000:
        blo, bhi, coff, csz = chunks.pop()
        mid = blo + 1
        while BLK_BASE[mid] - BLK_BASE[blo] < csz // 2:
            mid += 1
        chunks.append((blo, mid, coff, int(BLK_BASE[mid] - BLK_BASE[blo])))
        chunks.append((mid, bhi, int(BLK_BASE[mid]), int(BLK_BASE[bhi] - BLK_BASE[mid])))
    return chunks


CHUNKS = _chunks()
CMAX = max(c[3] for c in CHUNKS)

_NC_CACHE = {}
LAST = {}


def _build_nc():
    import bass_rust
    import concourse.bacc as bacc
    import concourse.bass as bass
    import concourse.mybir as mybir
    import concourse.tile as tile

    def _sub_ap(t, off, dims):
        return bass.AP(tensor=t.tensor, offset=t.offset + off, ap=[t.ap[0]] + dims)

    f32 = mybir.dt.float32
    bf16 = mybir.dt.bfloat16
    nc = bacc.Bacc()

    inp16_d = nc.declare_dram_parameter("inp16", [H, IC16], bf16, isOutput=False)
    # f32 side data: col 0 = fcb (rows 0:384), col 1 = zeros
    aux_d = nc.declare_dram_parameter("aux", [H, 2], f32, isOutput=False)
    out_d = nc.declare_dram_parameter("out", [OC, 128, PPAD], bf16, isOutput=True)

    Tanh = mybir.ActivationFunctionType.Tanh

    with tile.TileContext(nc) as tc:
        with (
            tc.tile_pool(name="const", bufs=1) as cpool,
            tc.tile_pool(name="mm", bufs=3, space="PSUM") as mpool,
            tc.tile_pool(name="outp", bufs=6) as opool,
            tc.tile_pool(name="outp2", bufs=6) as opool2,
        ):
            inp_b = cpool.tile([128, KT * IC16], bf16, name="inp_b")
            inp_r = inp_b[:].rearrange("p (t c) -> p t c", t=KT)
            src_r = inp16_d.rearrange("(t p) c -> p t c", p=128)
            # part A: ht + stripe-0 weights, one k-tile per DMA, issues
            # alternating between the SP and DVE sequencers (a single
            # sequencer serializes issues ~0.7us apart and stalls PE)
            for kk in range(KT):
                eng = nc.sync if kk % 2 == 0 else nc.scalar
                eng.dma_start(
                    inp_r[:, kk : kk + 1, 0:512], src_r[:, kk : kk + 1, 0:512]
                )
            aux_b = cpool.tile([128, KT * 2], f32, name="aux_b")
            nc.sync.dma_start(
                aux_b[:].rearrange("p (t c) -> p t c", t=KT),
                aux_d.rearrange("(t p) c -> p t c", p=128),
            )
            # part B: stripe 1-2 weights, one DMA
            nc.sync.dma_start(
                inp_r[:, :, 512:IC16], src_r[:, :, 512:IC16]
            )

            ht_t = [inp_b[:, kk * IC16 : kk * IC16 + S] for kk in range(KT)]
            fcb_t = [aux_b[:, c * 2 : c * 2 + 1] for c in range(OC)]

            prev_stops = []
            for c in range(OC):
                w1c = S + 256 * c
                w2c = S + 256 * c + 128
                pm1 = mpool.tile([128, S], f32, name="pm1")
                pm2 = mpool.tile([128, S], f32, name="pm2")
                stops = []
                for pm, wc in ((pm1, w1c), (pm2, w2c)):
                    for kk in range(KT):
                        mm = nc.tensor.matmul(
                            pm[:, :S],
                            inp_b[:, kk * IC16 + wc : kk * IC16 + wc + 128],
                            ht_t[kk],
                            start=(kk == 0),
                            stop=(kk == KT - 1),
                        )
                        if kk == 0 and prev_stops:
                            # keep PE stripe-major: without this the scheduler
                            # interleaves all stripes k-tile-major and stripe
                            # 0's PSUM stop retires ~5us late
                            deps = bass_rust.InstructionNameOrderedSet()
                            for nm in prev_stops:
                                deps.add(nm)
                            mm.ins.add_nosync_dependencies_from(deps)
                        if kk == KT - 1:
                            stops.append(mm.ins.name)
                prev_stops = stops

                p1 = cpool.tile([128, S], bf16, name=f"p1_{c}")
                q2 = cpool.tile([128, S + GPAD], bf16, name=f"q2_{c}")
                nc.vector.memset(q2[:, S : S + GPAD], 0.0)
                nc.vector.tensor_copy(p1[:], pm1[:])
                nc.vector.tensor_scalar_add(q2[:, :S], pm2[:], fcb_t[c])
                for ci, (blo, bhi, coff, csz) in enumerate(CHUNKS):
                    ot = opool.tile([128, CMAX], bf16, name="ot")
                    for bb in range(blo, bhi):
                        d0, G, L = BLOCKS[bb]
                        off = int(BLK_BASE[bb]) - coff
                        nc.vector.tensor_tensor(
                            _sub_ap(ot, off, [[L, G], [1, L]]),
                            _sub_ap(p1, 0, [[0, G], [1, L]]),
                            _sub_ap(q2, d0, [[1, G], [1, L]]),
                            op=mybir.AluOpType.add,
                        )
                    ot2 = opool2.tile([128, CMAX], bf16, name="ot2")
                    nc.scalar.activation(ot2[:, :csz], ot[:, :csz], Tanh)
                    # the final three DMAs issue from three different
                    # sequencers (ACT/Pool idle by then) so their issue+DGE
                    # latencies overlap instead of stacking on SP's queue
                    dma_eng = nc.sync
                    if c == OC - 1:
                        if ci == len(CHUNKS) - 1:
                            dma_eng = nc.scalar
                    dma_eng.dma_start(out_d[c, :, coff : coff + csz], ot2[:, :csz])
    nc.compile()
    return nc


def _get_nc():
    if "nc" not in _NC_CACHE:
        _NC_CACHE["nc"] = _build_nc()
    return _NC_CACHE["nc"]


def _make_in_maps(hidden_state, fc_w, fc_b):
    import ml_dtypes

    in_maps = []
    for k in range(8):
        b, h0 = k // 2, 384 * (k % 2)
        inp16 = np.empty((H, IC16), dtype=ml_dtypes.bfloat16)
        inp16[:, :S] = hidden_state[b].T.astype(ml_dtypes.bfloat16)
        for c in range(OC):
            r0 = h0 + 128 * c
            inp16[:, S + 256 * c : S + 256 * c + 128] = fc_w[
                r0 : r0 + 128, :H
            ].T.astype(ml_dtypes.bfloat16)
            inp16[:, S + 256 * c + 128 : S + 256 * c + 256] = fc_w[
                r0 : r0 + 128, H:
            ].T.astype(ml_dtypes.bfloat16)
        aux = np.zeros((H, 2), dtype=np.float32)
        aux[: 128 * OC, 0] = fc_b[h0 : h0 + 384]
        in_maps.append(dict(inp16=inp16, aux=aux))
    return in_maps


def _devcol():
    """Map triu pair index p -> device (diagonal-major padded) column."""
    colstart = np.empty(S, dtype=np.int64)
    for bi, (d0, G, L) in enumerate(BLOCKS):
        for g in range(G):
            colstart[d0 + g] = BLK_BASE[bi] + g * L
    ii, jj = np.triu_indices(S)
    return colstart[jj - ii] + ii


_DEVCOL = _devcol()


def kernel(hidden_state, fc_w, fc_b, _trace=False, **_trace_kwargs):
    from concourse.bass_utils import run_bass_kernel_spmd

    hidden_state = np.asarray(hidden_state, dtype=np.float32)
    fc_w = np.asarray(fc_w, dtype=np.float32)
    fc_b = np.asarray(fc_b, dtype=np.float32)

    in_maps = _make_in_maps(hidden_state, fc_w, fc_b)
    nc = _get_nc()
    res = run_bass_kernel_spmd(
        nc, in_maps, core_ids=list(range(8)), trace=_trace, **_trace_kwargs
    )
    LAST["res"] = res

    full = np.empty((B, H, P), dtype=np.float32)
    for k in range(8):
        b, h0 = k // 2, 384 * (k % 2)
        dev = res.results[k]["out"].reshape(384, PPAD)
        full[b, h0 : h0 + 384] = dev[:, _DEVCOL].astype(np.float32)
    return np.ascontiguousarray(full.transpose(0, 2, 1))
